# revision 3
# baseline (speedup 1.0000x reference)
"""DistSageConv Trainium2 kernel: SAGE mean-aggregation GNN over 8 NeuronCores.

Strategy: nodes (and their in-edges) are sharded by dst across 8 cores; feat is
replicated in each core's HBM. Each core gathers feat[src] for its edges with
dma_gather (int16 indices -> 4 source banks of 25000 rows), segment-sums them
into its 12500 output nodes via one-hot matmuls on the PE (edges pre-bucketed
by 128-dst windows on host), and applies the dense projections:

    out = feat @ W_self.T + (agg/deg) @ W_neigh.T + b

The per-(window,bank) edge-group sizes are padded to the max across the 8
cores so a single compiled program (SPMD) serves every core.
"""

import numpy as np
from dataclasses import dataclass


@dataclass(frozen=True)
class Cfg:
    N: int = 100000          # nodes
    D: int = 64              # feature dim (in == out)
    C: int = 8               # cores
    WIN: int = 128           # dst window (one-hot width)
    NBANK: int = 4           # src banks (int16 gather index limit)
    PHASE_WINS: tuple = (3,) * 32 + (2,)   # windows per phase (sum == NW)

    @property
    def NPC(self):  # nodes per core
        return self.N // self.C

    @property
    def NW(self):   # windows per core
        return -(-self.NPC // self.WIN)

    @property
    def BROWS(self):  # rows per src bank
        return self.N // self.NBANK


CFG = Cfg()
SENTINEL = 1000.0  # dst-in-window value for padding edges (never matches iota)
PSUM_WSTRIDE = 512  # f32 elems between windows in psum (= one 2KB zero region)


@dataclass
class Structure:
    """Shared (core-independent) program structure."""
    TC: int                      # total chunk slots
    E_struct: int                # padded edge count (TC*128)
    slot_w: np.ndarray           # [TC] global window of each slot
    phase_slot0: list            # per phase: first slot
    phase_slot1: list            # per phase: end slot
    phase_w0: list               # per phase: first window
    phase_nw: list               # per phase: #windows
    calls: list                  # per phase: list of (bank, slot0, nslots, idxcol0)


@dataclass
class CoreData:
    idx_blob: np.ndarray     # [128, E_struct//16] int16
    dstloc: np.ndarray       # [128, TC] f32
    invdegb: np.ndarray      # [D, NW*WIN] f32 (row-replicated 1/deg)
    featT_aug: np.ndarray    # [D+1, NW*WIN] f32  (row D = ones)


def preprocess(feat, src, dst, W_self, W_neigh, b, cfg: Cfg = CFG):
    N, D, C = cfg.N, cfg.D, cfg.C
    NPC, WIN, NW = cfg.NPC, cfg.WIN, cfg.NW
    NBANK, BROWS = cfg.NBANK, cfg.BROWS
    phases = list(cfg.PHASE_WINS)
    NP = len(phases)
    assert sum(phases) == NW

    feat = np.asarray(feat, dtype=np.float32)
    src32 = np.asarray(src).astype(np.int64)
    dst32 = np.asarray(dst).astype(np.int64)
    E = src32.shape[0]
    W_self = np.asarray(W_self, dtype=np.float32)
    W_neigh = np.asarray(W_neigh, dtype=np.float32)
    b = np.asarray(b, dtype=np.float32)

    deg = np.bincount(dst32, minlength=N).astype(np.float32)
    invdeg = (1.0 / np.maximum(deg, 1.0)).astype(np.float32)

    core = dst32 // NPC
    dloc = dst32 - core * NPC
    wl = dloc // WIN                      # window within core
    dwin = (dloc - wl * WIN).astype(np.float32)
    bank = src32 // BROWS
    lidx = (src32 - bank * BROWS).astype(np.int16)

    counts = np.bincount(
        (core * NW + wl) * NBANK + bank, minlength=C * NW * NBANK
    ).reshape(C, NW, NBANK)
    chunks_wb = -(-counts.max(axis=0) // 128)  # [NW, NBANK] ceil

    w0_of_p = np.concatenate([[0], np.cumsum(phases)])[:-1]

    # groups ordered (phase, bank, window); each (phase,bank) run is one
    # dma_gather call writing contiguous arena slots.
    group_list = []
    for p in range(NP):
        for bb in range(NBANK):
            for w in range(w0_of_p[p], w0_of_p[p] + phases[p]):
                if chunks_wb[w, bb] > 0:
                    group_list.append((p, bb, w))
    G = len(group_list)
    gidx = np.full((NW, NBANK), -1, dtype=np.int64)
    for gi, (p, bb, w) in enumerate(group_list):
        gidx[w, bb] = gi
    gchunks = np.array([chunks_wb[w, bb] for (p, bb, w) in group_list])
    gcap = gchunks * 128
    gbase = np.concatenate([[0], np.cumsum(gcap)])[:-1]
    E_struct = int(gcap.sum())
    TC = E_struct // 128

    # padded position of each edge within its core's layout
    gid_e = gidx[wl, bank]
    assert (gid_e >= 0).all()
    keyv = core * G + gid_e
    order = np.argsort(keyv, kind="stable")
    ks = keyv[order]
    starts = np.concatenate([[0], np.flatnonzero(ks[1:] != ks[:-1]) + 1])
    runlen = np.diff(np.concatenate([starts, [E]]))
    rank = np.empty(E, np.int64)
    rank[order] = np.arange(E) - np.repeat(starts, runlen)
    assert (rank < gcap[gid_e]).all()
    pos = gbase[gid_e] + rank

    # slot metadata
    slot_w = np.repeat(np.array([w for (_, _, w) in group_list]), gchunks)
    slot_p = np.repeat(np.array([p for (p, _, _) in group_list]), gchunks)
    phase_slot0 = [int(np.searchsorted(slot_p, p, "left")) for p in range(NP)]
    phase_slot1 = [int(np.searchsorted(slot_p, p, "right")) for p in range(NP)]

    # gather calls: per (phase, bank) contiguous group runs
    calls = [[] for _ in range(NP)]
    idxcol = 0
    gslot = np.concatenate([[0], np.cumsum(gchunks)])  # slot base per group
    gi = 0
    while gi < G:
        p, bb, _ = group_list[gi]
        gj = gi
        while gj < G and group_list[gj][0] == p and group_list[gj][1] == bb:
            gj += 1
        s0, s1 = int(gslot[gi]), int(gslot[gj])
        nslots = s1 - s0
        calls[p].append((bb, s0, nslots, idxcol))
        idxcol += nslots * 8  # nslots*128/16 columns
        gi = gj
    assert idxcol == E_struct // 16

    st = Structure(
        TC=TC, E_struct=E_struct, slot_w=slot_w,
        phase_slot0=phase_slot0, phase_slot1=phase_slot1,
        phase_w0=[int(x) for x in w0_of_p], phase_nw=phases,
        calls=calls,
    )

    # per-core staged data
    lidx_pad = np.zeros((C, E_struct), np.int16)
    dwin_pad = np.full((C, E_struct), SENTINEL, np.float32)
    lidx_pad[core, pos] = lidx
    dwin_pad[core, pos] = dwin

    cores = []
    for k in range(C):
        blocks = []
        for p in range(NP):
            for (bb, s0, nslots, _c0) in calls[p]:
                seg = lidx_pad[k, s0 * 128:(s0 + nslots) * 128]
                blocks.append(seg.reshape(-1, 16).T)  # [16, nslots*8]
        blob16 = np.concatenate(blocks, axis=1)
        idx_blob = np.tile(blob16, (8, 1))  # [128, E_struct//16]

        dstloc = dwin_pad[k].reshape(TC, 128).T.copy()  # [128, TC]

        v = np.ones(NW * WIN, np.float32)
        v[:NPC] = invdeg[k * NPC:(k + 1) * NPC]
        invdegb = np.tile(v, (D, 1))  # [D, NW*WIN]

        ft = np.zeros((D + 1, NW * WIN), np.float32)
        ft[:D, :NPC] = feat[k * NPC:(k + 1) * NPC].T
        ft[D, :] = 1.0
        cores.append(CoreData(idx_blob=idx_blob, dstloc=dstloc,
                              invdegb=invdegb, featT_aug=ft))

    consts = {
        "WselfT_aug": np.concatenate([W_self.T, b[None, :]], 0).astype(np.float32),
        "WneighT": np.ascontiguousarray(W_neigh.T, dtype=np.float32),
        "iota": np.tile(np.arange(128, dtype=np.float32), (128, 1)),
        "feat_full": np.ascontiguousarray(feat, dtype=np.float32),
    }
    return st, cores, consts


def build_program(st: Structure, cfg: Cfg = CFG):
    import concourse.bacc as bacc
    import concourse.mybir as mybir
    import concourse.tile as tile

    D, WIN, NW = cfg.D, cfg.WIN, cfg.NW
    NP = len(cfg.PHASE_WINS)
    f32 = mybir.dt.float32
    AL = mybir.AluOpType

    nc = bacc.Bacc("TRN2", target_bir_lowering=False, debug=False)

    feat_d = nc.dram_tensor("feat", [cfg.N, D], f32, kind="ExternalInput")
    featT_d = nc.dram_tensor("featT", [D + 1, NW * WIN], f32, kind="ExternalInput")
    idx_d = nc.dram_tensor("idx", [128, st.E_struct // 16], mybir.dt.int16,
                           kind="ExternalInput")
    dstloc_d = nc.dram_tensor("dstloc", [128, st.TC], f32, kind="ExternalInput")
    invdegb_d = nc.dram_tensor("invdegb", [D, NW * WIN], f32, kind="ExternalInput")
    wself_d = nc.dram_tensor("WselfT_aug", [D + 1, D], f32, kind="ExternalInput")
    wneigh_d = nc.dram_tensor("WneighT", [D, D], f32, kind="ExternalInput")
    iota_d = nc.dram_tensor("iota", [128, 128], f32, kind="ExternalInput")
    out_d = nc.dram_tensor("out", [NW * WIN, D], f32, kind="ExternalOutput")

    with tile.TileContext(nc) as tc:
        with (
            tc.tile_pool(name="const", bufs=1) as cpool,
            tc.tile_pool(name="arena", bufs=2) as apool,
            tc.tile_pool(name="mask", bufs=2) as mpool,
            tc.tile_pool(name="featT", bufs=2) as fpool,
            tc.tile_pool(name="aggT", bufs=2) as gpool,
            tc.tile_pool(name="invd", bufs=2) as ipool,
            tc.tile_pool(name="outb", bufs=2) as opool,
            tc.tile_pool(name="psA", bufs=2, space="PSUM") as psA,
            tc.tile_pool(name="psO", bufs=2, space="PSUM") as psO,
        ):
            idx_sb = cpool.tile([128, st.E_struct // 16], mybir.dt.int16, tag="idx")
            nc.sync.dma_start(idx_sb[:], idx_d[:, :])
            dstloc_sb = cpool.tile([128, st.TC], f32, tag="dstloc")
            nc.sync.dma_start(dstloc_sb[:], dstloc_d[:, :])
            iota_sb = cpool.tile([128, 128], f32, tag="iota")
            nc.sync.dma_start(iota_sb[:], iota_d[:, :])
            wself_sb = cpool.tile([D + 1, D], f32, tag="wself")
            nc.sync.dma_start(wself_sb[:], wself_d[:, :])
            wneigh_sb = cpool.tile([D, D], f32, tag="wneigh")
            nc.sync.dma_start(wneigh_sb[:], wneigh_d[:, :])

            for p in range(NP):
                ps0, ps1 = st.phase_slot0[p], st.phase_slot1[p]
                tcp = ps1 - ps0          # chunks this phase
                w0, nwv = st.phase_w0[p], st.phase_nw[p]
                n0 = w0 * WIN

                arena = apool.tile([128, tcp, D], f32, tag="arena")
                for (bb, s0, nslots, c0) in st.calls[p]:
                    nc.gpsimd.dma_gather(
                        arena[:, s0 - ps0:s0 - ps0 + nslots, :],
                        feat_d[bb * cfg.BROWS:(bb + 1) * cfg.BROWS, :],
                        idx_sb[:, c0:c0 + nslots * 8],
                        nslots * 128,
                        nslots * 128,
                        D,
                        single_packet=False,
                    )

                featT_t = fpool.tile([D + 1, nwv * WIN], f32, tag="featT")
                nc.sync.dma_start(featT_t[:], featT_d[:, n0:n0 + nwv * WIN])
                invd_t = ipool.tile([D, nwv * WIN], f32, tag="invd")
                nc.sync.dma_start(invd_t[:], invdegb_d[:, n0:n0 + nwv * WIN])

                # one-hot masks for the whole phase in one DVE op
                mask = mpool.tile([128, tcp, 128], f32, tag="mask")
                nc.vector.tensor_tensor(
                    mask[:],
                    iota_sb[:].unsqueeze(1).to_broadcast([128, tcp, 128]),
                    dstloc_sb[:, ps0:ps1].unsqueeze(2)
                    .to_broadcast([128, tcp, 128]),
                    AL.is_equal,
                )

                # window-major matmul issue; each window owns one 2KB psum
                # zero region (stride PSUM_WSTRIDE)
                psum_agg = psA.tile([D, nwv * PSUM_WSTRIDE], f32, tag="psA")
                for wl in range(nwv):
                    w = w0 + wl
                    slots = [s for s in range(ps0, ps1) if st.slot_w[s] == w]
                    for i, s in enumerate(slots):
                        nc.tensor.matmul(
                            psum_agg[:, wl * PSUM_WSTRIDE:
                                     wl * PSUM_WSTRIDE + WIN],
                            lhsT=arena[:, s - ps0, :],
                            rhs=mask[:, s - ps0, :],
                            start=(i == 0),
                            stop=(i == len(slots) - 1),
                        )

                # psum -> sbuf with mean normalization: aggT = agg * invdeg
                aggT = gpool.tile([D, nwv * WIN], f32, tag="aggT")
                nc.vector.scalar_tensor_tensor(
                    aggT[:].rearrange("f (w d) -> f w d", d=WIN),
                    psum_agg[:].rearrange("f (w d) -> f w d", d=PSUM_WSTRIDE)
                    [:, :, :WIN],
                    1.0,
                    invd_t[:].rearrange("f (w d) -> f w d", d=WIN),
                    AL.mult,
                    AL.mult,
                )

                outb = opool.tile([128, nwv, D], f32, tag="outb")
                for wl in range(nwv):
                    psum_o = psO.tile([128, D], f32, tag="psO")
                    nc.tensor.matmul(
                        psum_o[:],
                        lhsT=featT_t[:, wl * WIN:(wl + 1) * WIN],
                        rhs=wself_sb[:],
                        start=True, stop=False,
                    )
                    nc.tensor.matmul(
                        psum_o[:],
                        lhsT=aggT[:, wl * WIN:(wl + 1) * WIN],
                        rhs=wneigh_sb[:],
                        start=False, stop=True,
                    )
                    nc.scalar.copy(outb[:, wl, :], psum_o[:])
                nc.sync.dma_start(
                    out_d[n0:n0 + nwv * WIN, :]
                    .rearrange("(w p) d -> p w d", p=128),
                    outb[:],
                )

    nc.compile()
    return nc


def run(feat, src, dst, W_self, W_neigh, b, cfg: Cfg = CFG, trace=False):
    from concourse.bass_utils import run_bass_kernel_spmd

    st, cores, consts = preprocess(feat, src, dst, W_self, W_neigh, b, cfg)
    nc = build_program(st, cfg)

    in_maps = []
    for k in range(cfg.C):
        cd = cores[k]
        in_maps.append({
            "feat": consts["feat_full"],
            "featT": cd.featT_aug,
            "idx": cd.idx_blob,
            "dstloc": cd.dstloc,
            "invdegb": cd.invdegb,
            "WselfT_aug": consts["WselfT_aug"],
            "WneighT": consts["WneighT"],
            "iota": consts["iota"],
        })
    res = run_bass_kernel_spmd(nc, in_maps, core_ids=list(range(cfg.C)),
                               trace=trace)
    out = np.empty((cfg.N, cfg.D), np.float32)
    for k in range(cfg.C):
        out[k * cfg.NPC:(k + 1) * cfg.NPC] = res.results[k]["out"][:cfg.NPC]
    return out, res


def kernel(feat, src, dst, W_self, W_neigh, b):
    out, _ = run(feat, src, dst, W_self, W_neigh, b)
    return out


# revision 5
# speedup vs baseline: 2.1375x; 2.1375x over previous
"""DistSageConv Trainium2 kernel: SAGE mean-aggregation GNN over 8 NeuronCores.

Strategy: nodes (and their in-edges) are sharded by dst across 8 cores; feat is
replicated in each core's HBM. Each core gathers feat[src] for its edges with
dma_gather (int16 indices -> 4 source banks of 25000 rows), segment-sums them
into its 12500 output nodes via one-hot matmuls on the PE (edges pre-bucketed
by 128-dst windows on host), and applies the dense projections:

    out = feat @ W_self.T + (agg/deg) @ W_neigh.T + b

The per-(window,bank) edge-group sizes are padded to the max across the 8
cores so a single compiled program (SPMD) serves every core.
"""

import numpy as np
from dataclasses import dataclass


@dataclass(frozen=True)
class Cfg:
    N: int = 100000          # nodes
    D: int = 64              # feature dim (in == out)
    C: int = 8               # cores
    WIN: int = 128           # dst window (one-hot width)
    NBANK: int = 4           # src banks (int16 gather index limit)
    PHASE_WINS: tuple = (3,) * 32 + (2,)   # windows per phase (sum == NW)

    @property
    def NPC(self):  # nodes per core
        return self.N // self.C

    @property
    def NW(self):   # windows per core
        return -(-self.NPC // self.WIN)

    @property
    def BROWS(self):  # rows per src bank
        return self.N // self.NBANK


CFG = Cfg()
SENTINEL = 1000.0  # dst-in-window value for padding edges (never matches iota)
PSUM_WSTRIDE = 512  # f32 elems between windows in psum (= one 2KB zero region)


@dataclass
class Structure:
    """Shared (core-independent) program structure."""
    TC: int                      # total chunk slots
    E_struct: int                # padded edge count (TC*128)
    slot_w: np.ndarray           # [TC] global window of each slot
    phase_slot0: list            # per phase: first slot
    phase_slot1: list            # per phase: end slot
    phase_w0: list               # per phase: first window
    phase_nw: list               # per phase: #windows
    calls: list                  # per phase: list of (bank, slot0, nslots, idxcol0)


@dataclass
class CoreData:
    idx_blob: np.ndarray     # [128, E_struct//16] int16
    dstloc: np.ndarray       # [128, TC] f32
    invdegb: np.ndarray      # [D, NW*WIN] f32 (row-replicated 1/deg)
    featT_aug: np.ndarray    # [D+1, NW*WIN] f32  (row D = ones)


def preprocess(feat, src, dst, W_self, W_neigh, b, cfg: Cfg = CFG):
    N, D, C = cfg.N, cfg.D, cfg.C
    NPC, WIN, NW = cfg.NPC, cfg.WIN, cfg.NW
    NBANK, BROWS = cfg.NBANK, cfg.BROWS
    phases = list(cfg.PHASE_WINS)
    NP = len(phases)
    assert sum(phases) == NW

    feat = np.asarray(feat, dtype=np.float32)
    src32 = np.asarray(src).astype(np.int64)
    dst32 = np.asarray(dst).astype(np.int64)
    E = src32.shape[0]
    W_self = np.asarray(W_self, dtype=np.float32)
    W_neigh = np.asarray(W_neigh, dtype=np.float32)
    b = np.asarray(b, dtype=np.float32)

    deg = np.bincount(dst32, minlength=N).astype(np.float32)
    invdeg = (1.0 / np.maximum(deg, 1.0)).astype(np.float32)

    core = dst32 // NPC
    dloc = dst32 - core * NPC
    wl = dloc // WIN                      # window within core
    dwin = (dloc - wl * WIN).astype(np.float32)
    bank = src32 // BROWS
    lidx = (src32 - bank * BROWS).astype(np.int16)

    counts = np.bincount(
        (core * NW + wl) * NBANK + bank, minlength=C * NW * NBANK
    ).reshape(C, NW, NBANK)
    chunks_wb = -(-counts.max(axis=0) // 128)  # [NW, NBANK] ceil

    w0_of_p = np.concatenate([[0], np.cumsum(phases)])[:-1]

    # groups ordered (phase, bank, window); each (phase,bank) run is one
    # dma_gather call writing contiguous arena slots.
    group_list = []
    for p in range(NP):
        for bb in range(NBANK):
            for w in range(w0_of_p[p], w0_of_p[p] + phases[p]):
                if chunks_wb[w, bb] > 0:
                    group_list.append((p, bb, w))
    G = len(group_list)
    gidx = np.full((NW, NBANK), -1, dtype=np.int64)
    for gi, (p, bb, w) in enumerate(group_list):
        gidx[w, bb] = gi
    gchunks = np.array([chunks_wb[w, bb] for (p, bb, w) in group_list])
    gcap = gchunks * 128
    gbase = np.concatenate([[0], np.cumsum(gcap)])[:-1]
    E_struct = int(gcap.sum())
    TC = E_struct // 128

    # padded position of each edge within its core's layout
    gid_e = gidx[wl, bank]
    assert (gid_e >= 0).all()
    keyv = core * G + gid_e
    order = np.argsort(keyv, kind="stable")
    ks = keyv[order]
    starts = np.concatenate([[0], np.flatnonzero(ks[1:] != ks[:-1]) + 1])
    runlen = np.diff(np.concatenate([starts, [E]]))
    rank = np.empty(E, np.int64)
    rank[order] = np.arange(E) - np.repeat(starts, runlen)
    assert (rank < gcap[gid_e]).all()
    pos = gbase[gid_e] + rank

    # slot metadata
    slot_w = np.repeat(np.array([w for (_, _, w) in group_list]), gchunks)
    slot_p = np.repeat(np.array([p for (p, _, _) in group_list]), gchunks)
    phase_slot0 = [int(np.searchsorted(slot_p, p, "left")) for p in range(NP)]
    phase_slot1 = [int(np.searchsorted(slot_p, p, "right")) for p in range(NP)]

    # gather calls: per (phase, bank) contiguous group runs
    calls = [[] for _ in range(NP)]
    idxcol = 0
    gslot = np.concatenate([[0], np.cumsum(gchunks)])  # slot base per group
    gi = 0
    while gi < G:
        p, bb, _ = group_list[gi]
        gj = gi
        while gj < G and group_list[gj][0] == p and group_list[gj][1] == bb:
            gj += 1
        s0, s1 = int(gslot[gi]), int(gslot[gj])
        nslots = s1 - s0
        calls[p].append((bb, s0, nslots, idxcol))
        idxcol += nslots * 8  # nslots*128/16 columns
        gi = gj
    assert idxcol == E_struct // 16

    st = Structure(
        TC=TC, E_struct=E_struct, slot_w=slot_w,
        phase_slot0=phase_slot0, phase_slot1=phase_slot1,
        phase_w0=[int(x) for x in w0_of_p], phase_nw=phases,
        calls=calls,
    )

    # per-core staged data
    lidx_pad = np.zeros((C, E_struct), np.int16)
    dwin_pad = np.full((C, E_struct), SENTINEL, np.float32)
    lidx_pad[core, pos] = lidx
    dwin_pad[core, pos] = dwin

    cores = []
    for k in range(C):
        blocks = []
        for p in range(NP):
            for (bb, s0, nslots, _c0) in calls[p]:
                seg = lidx_pad[k, s0 * 128:(s0 + nslots) * 128]
                blocks.append(seg.reshape(-1, 16).T)  # [16, nslots*8]
        blob16 = np.concatenate(blocks, axis=1)
        idx_blob = np.tile(blob16, (8, 1))  # [128, E_struct//16]

        dstloc = dwin_pad[k].reshape(TC, 128).T.copy()  # [128, TC]

        v = np.ones(NW * WIN, np.float32)
        v[:NPC] = invdeg[k * NPC:(k + 1) * NPC]
        invdegb = np.tile(v, (D, 1))  # [D, NW*WIN]

        ft = np.zeros((D + 1, NW * WIN), np.float32)
        ft[:D, :NPC] = feat[k * NPC:(k + 1) * NPC].T
        ft[D, :] = 1.0
        cores.append(CoreData(idx_blob=idx_blob, dstloc=dstloc,
                              invdegb=invdegb, featT_aug=ft))

    consts = {
        "WselfT_aug": np.concatenate([W_self.T, b[None, :]], 0).astype(np.float32),
        "WneighT": np.ascontiguousarray(W_neigh.T, dtype=np.float32),
        "iota": np.tile(np.arange(128, dtype=np.float32), (128, 1)),
        "feat_full": np.ascontiguousarray(feat, dtype=np.float32),
    }
    return st, cores, consts


def build_program(st: Structure, cfg: Cfg = CFG):
    import concourse.bacc as bacc
    import concourse.mybir as mybir
    import concourse.tile as tile

    D, WIN, NW = cfg.D, cfg.WIN, cfg.NW
    NP = len(cfg.PHASE_WINS)
    f32 = mybir.dt.float32
    AL = mybir.AluOpType

    nc = bacc.Bacc("TRN2", target_bir_lowering=False, debug=False,
                   num_swdge_queues=4)

    feat_d = nc.dram_tensor("feat", [cfg.N, D], f32, kind="ExternalInput")
    featT_d = nc.dram_tensor("featT", [D + 1, NW * WIN], f32, kind="ExternalInput")
    idx_d = nc.dram_tensor("idx", [128, st.E_struct // 16], mybir.dt.int16,
                           kind="ExternalInput")
    dstloc_d = nc.dram_tensor("dstloc", [128, st.TC], f32, kind="ExternalInput")
    invdegb_d = nc.dram_tensor("invdegb", [D, NW * WIN], f32, kind="ExternalInput")
    wself_d = nc.dram_tensor("WselfT_aug", [D + 1, D], f32, kind="ExternalInput")
    wneigh_d = nc.dram_tensor("WneighT", [D, D], f32, kind="ExternalInput")
    iota_d = nc.dram_tensor("iota", [128, 128], f32, kind="ExternalInput")
    out_d = nc.dram_tensor("out", [NW * WIN, D], f32, kind="ExternalOutput")

    with tile.TileContext(nc) as tc:
        with (
            tc.tile_pool(name="const", bufs=1) as cpool,
            tc.tile_pool(name="arena", bufs=2) as apool,
            tc.tile_pool(name="mask", bufs=2) as mpool,
            tc.tile_pool(name="featT", bufs=2) as fpool,
            tc.tile_pool(name="aggT", bufs=2) as gpool,
            tc.tile_pool(name="invd", bufs=2) as ipool,
            tc.tile_pool(name="outb", bufs=2) as opool,
            tc.tile_pool(name="psA", bufs=2, space="PSUM") as psA,
            tc.tile_pool(name="psO", bufs=2, space="PSUM") as psO,
        ):
            idx_sb = cpool.tile([128, st.E_struct // 16], mybir.dt.int16, tag="idx")
            nc.sync.dma_start(idx_sb[:], idx_d[:, :])
            dstloc_sb = cpool.tile([128, st.TC], f32, tag="dstloc")
            nc.sync.dma_start(dstloc_sb[:], dstloc_d[:, :])
            iota_sb = cpool.tile([128, 128], f32, tag="iota")
            nc.sync.dma_start(iota_sb[:], iota_d[:, :])
            wself_sb = cpool.tile([D + 1, D], f32, tag="wself")
            nc.sync.dma_start(wself_sb[:], wself_d[:, :])
            wneigh_sb = cpool.tile([D, D], f32, tag="wneigh")
            nc.sync.dma_start(wneigh_sb[:], wneigh_d[:, :])

            qcounter = [0]
            for p in range(NP):
                ps0, ps1 = st.phase_slot0[p], st.phase_slot1[p]
                tcp = ps1 - ps0          # chunks this phase
                w0, nwv = st.phase_w0[p], st.phase_nw[p]
                n0 = w0 * WIN

                arena = apool.tile([128, tcp, D], f32, tag="arena")
                for (bb, s0, nslots, c0) in st.calls[p]:
                    nc.gpsimd.dma_gather(
                        arena[:, s0 - ps0:s0 - ps0 + nslots, :],
                        feat_d[bb * cfg.BROWS:(bb + 1) * cfg.BROWS, :],
                        idx_sb[:, c0:c0 + nslots * 8],
                        nslots * 128,
                        nslots * 128,
                        D,
                        single_packet=False,
                        queue_num=qcounter[0] % 4,
                    )
                    qcounter[0] += 1

                featT_t = fpool.tile([D + 1, nwv * WIN], f32, tag="featT")
                nc.sync.dma_start(featT_t[:], featT_d[:, n0:n0 + nwv * WIN])
                invd_t = ipool.tile([D, nwv * WIN], f32, tag="invd")
                nc.sync.dma_start(invd_t[:], invdegb_d[:, n0:n0 + nwv * WIN])

                # one-hot masks for the whole phase in one DVE op
                mask = mpool.tile([128, tcp, 128], f32, tag="mask")
                nc.vector.tensor_tensor(
                    mask[:],
                    iota_sb[:].unsqueeze(1).to_broadcast([128, tcp, 128]),
                    dstloc_sb[:, ps0:ps1].unsqueeze(2)
                    .to_broadcast([128, tcp, 128]),
                    AL.is_equal,
                )

                # window-major matmul issue; each window owns one 2KB psum
                # zero region (stride PSUM_WSTRIDE)
                psum_agg = psA.tile([D, nwv * PSUM_WSTRIDE], f32, tag="psA")
                for wl in range(nwv):
                    w = w0 + wl
                    slots = [s for s in range(ps0, ps1) if st.slot_w[s] == w]
                    for i, s in enumerate(slots):
                        nc.tensor.matmul(
                            psum_agg[:, wl * PSUM_WSTRIDE:
                                     wl * PSUM_WSTRIDE + WIN],
                            lhsT=arena[:, s - ps0, :],
                            rhs=mask[:, s - ps0, :],
                            start=(i == 0),
                            stop=(i == len(slots) - 1),
                        )

                # psum -> sbuf with mean normalization: aggT = agg * invdeg
                aggT = gpool.tile([D, nwv * WIN], f32, tag="aggT")
                nc.vector.scalar_tensor_tensor(
                    aggT[:].rearrange("f (w d) -> f w d", d=WIN),
                    psum_agg[:].rearrange("f (w d) -> f w d", d=PSUM_WSTRIDE)
                    [:, :, :WIN],
                    1.0,
                    invd_t[:].rearrange("f (w d) -> f w d", d=WIN),
                    AL.mult,
                    AL.mult,
                )

                outb = opool.tile([128, nwv, D], f32, tag="outb")
                for wl in range(nwv):
                    psum_o = psO.tile([128, D], f32, tag="psO")
                    nc.tensor.matmul(
                        psum_o[:],
                        lhsT=featT_t[:, wl * WIN:(wl + 1) * WIN],
                        rhs=wself_sb[:],
                        start=True, stop=False,
                    )
                    nc.tensor.matmul(
                        psum_o[:],
                        lhsT=aggT[:, wl * WIN:(wl + 1) * WIN],
                        rhs=wneigh_sb[:],
                        start=False, stop=True,
                    )
                    nc.scalar.copy(outb[:, wl, :], psum_o[:])
                nc.sync.dma_start(
                    out_d[n0:n0 + nwv * WIN, :]
                    .rearrange("(w p) d -> p w d", p=128),
                    outb[:],
                )

    nc.compile()
    return nc


def run(feat, src, dst, W_self, W_neigh, b, cfg: Cfg = CFG, trace=False):
    from concourse.bass_utils import run_bass_kernel_spmd

    st, cores, consts = preprocess(feat, src, dst, W_self, W_neigh, b, cfg)
    nc = build_program(st, cfg)

    in_maps = []
    for k in range(cfg.C):
        cd = cores[k]
        in_maps.append({
            "feat": consts["feat_full"],
            "featT": cd.featT_aug,
            "idx": cd.idx_blob,
            "dstloc": cd.dstloc,
            "invdegb": cd.invdegb,
            "WselfT_aug": consts["WselfT_aug"],
            "WneighT": consts["WneighT"],
            "iota": consts["iota"],
        })
    res = run_bass_kernel_spmd(nc, in_maps, core_ids=list(range(cfg.C)),
                               trace=trace)
    out = np.empty((cfg.N, cfg.D), np.float32)
    for k in range(cfg.C):
        out[k * cfg.NPC:(k + 1) * cfg.NPC] = res.results[k]["out"][:cfg.NPC]
    return out, res


def kernel(feat, src, dst, W_self, W_neigh, b):
    out, _ = run(feat, src, dst, W_self, W_neigh, b)
    return out


# revision 6
# speedup vs baseline: 2.5221x; 1.1800x over previous
"""DistSageConv Trainium2 kernel: SAGE mean-aggregation GNN over 8 NeuronCores.

Strategy: nodes (and their in-edges) are sharded by dst across 8 cores; feat is
replicated in each core's HBM. Each core gathers feat[src] for its edges with
dma_gather (int16 indices -> 4 source banks of 25000 rows), segment-sums them
into its 12500 output nodes via one-hot matmuls on the PE (edges pre-bucketed
by 128-dst windows on host), and applies the dense projections:

    out = feat @ W_self.T + (agg/deg) @ W_neigh.T + b

The per-(window,bank) edge-group sizes are padded to the max across the 8
cores so a single compiled program (SPMD) serves every core.
"""

import numpy as np
from dataclasses import dataclass


@dataclass(frozen=True)
class Cfg:
    N: int = 100000          # nodes
    D: int = 64              # feature dim (in == out)
    C: int = 8               # cores
    WIN: int = 128           # dst window (one-hot width)
    NBANK: int = 4           # src banks (int16 gather index limit)
    PHASE_WINS: tuple = (3,) * 32 + (2,)   # windows per phase (sum == NW)

    @property
    def NPC(self):  # nodes per core
        return self.N // self.C

    @property
    def NW(self):   # windows per core
        return -(-self.NPC // self.WIN)

    @property
    def BROWS(self):  # rows per src bank
        return self.N // self.NBANK


CFG = Cfg()
SENTINEL = 1000.0  # dst-in-window value for padding edges (never matches iota)
PSUM_WSTRIDE = 512  # f32 elems between windows in psum (= one 2KB zero region)


@dataclass
class Structure:
    """Shared (core-independent) program structure."""
    TC: int                      # total chunk slots
    E_struct: int                # padded edge count (TC*128)
    slot_w: np.ndarray           # [TC] global window of each slot
    phase_slot0: list            # per phase: first slot
    phase_slot1: list            # per phase: end slot
    phase_w0: list               # per phase: first window
    phase_nw: list               # per phase: #windows
    calls: list                  # per phase: list of (bank, slot0, nslots, idxcol0)


@dataclass
class CoreData:
    idx_blob: np.ndarray     # [128, E_struct//16] int16
    dstloc: np.ndarray       # [128, TC] f32
    invdegb: np.ndarray      # [D, NW*WIN] f32 (row-replicated 1/deg)
    featT_aug: np.ndarray    # [D+1, NW*WIN] f32  (row D = ones)


def preprocess(feat, src, dst, W_self, W_neigh, b, cfg: Cfg = CFG):
    N, D, C = cfg.N, cfg.D, cfg.C
    NPC, WIN, NW = cfg.NPC, cfg.WIN, cfg.NW
    NBANK, BROWS = cfg.NBANK, cfg.BROWS
    phases = list(cfg.PHASE_WINS)
    NP = len(phases)
    assert sum(phases) == NW

    feat = np.asarray(feat, dtype=np.float32)
    src32 = np.asarray(src).astype(np.int64)
    dst32 = np.asarray(dst).astype(np.int64)
    E = src32.shape[0]
    W_self = np.asarray(W_self, dtype=np.float32)
    W_neigh = np.asarray(W_neigh, dtype=np.float32)
    b = np.asarray(b, dtype=np.float32)

    deg = np.bincount(dst32, minlength=N).astype(np.float32)
    invdeg = (1.0 / np.maximum(deg, 1.0)).astype(np.float32)

    core = dst32 // NPC
    dloc = dst32 - core * NPC
    wl = dloc // WIN                      # window within core
    dwin = (dloc - wl * WIN).astype(np.float32)
    bank = src32 // BROWS
    lidx = (src32 - bank * BROWS).astype(np.int16)

    counts = np.bincount(
        (core * NW + wl) * NBANK + bank, minlength=C * NW * NBANK
    ).reshape(C, NW, NBANK)
    chunks_wb = -(-counts.max(axis=0) // 128)  # [NW, NBANK] ceil

    w0_of_p = np.concatenate([[0], np.cumsum(phases)])[:-1]

    # groups ordered (phase, bank, window); each (phase,bank) run is one
    # dma_gather call writing contiguous arena slots.
    group_list = []
    for p in range(NP):
        for bb in range(NBANK):
            for w in range(w0_of_p[p], w0_of_p[p] + phases[p]):
                if chunks_wb[w, bb] > 0:
                    group_list.append((p, bb, w))
    G = len(group_list)
    gidx = np.full((NW, NBANK), -1, dtype=np.int64)
    for gi, (p, bb, w) in enumerate(group_list):
        gidx[w, bb] = gi
    gchunks = np.array([chunks_wb[w, bb] for (p, bb, w) in group_list])
    gcap = gchunks * 128
    gbase = np.concatenate([[0], np.cumsum(gcap)])[:-1]
    E_struct = int(gcap.sum())
    TC = E_struct // 128

    # padded position of each edge within its core's layout
    gid_e = gidx[wl, bank]
    assert (gid_e >= 0).all()
    keyv = core * G + gid_e
    order = np.argsort(keyv, kind="stable")
    ks = keyv[order]
    starts = np.concatenate([[0], np.flatnonzero(ks[1:] != ks[:-1]) + 1])
    runlen = np.diff(np.concatenate([starts, [E]]))
    rank = np.empty(E, np.int64)
    rank[order] = np.arange(E) - np.repeat(starts, runlen)
    assert (rank < gcap[gid_e]).all()
    pos = gbase[gid_e] + rank

    # slot metadata
    slot_w = np.repeat(np.array([w for (_, _, w) in group_list]), gchunks)
    slot_p = np.repeat(np.array([p for (p, _, _) in group_list]), gchunks)
    phase_slot0 = [int(np.searchsorted(slot_p, p, "left")) for p in range(NP)]
    phase_slot1 = [int(np.searchsorted(slot_p, p, "right")) for p in range(NP)]

    # gather calls: per (phase, bank) contiguous group runs
    calls = [[] for _ in range(NP)]
    idxcol = 0
    gslot = np.concatenate([[0], np.cumsum(gchunks)])  # slot base per group
    gi = 0
    while gi < G:
        p, bb, _ = group_list[gi]
        gj = gi
        while gj < G and group_list[gj][0] == p and group_list[gj][1] == bb:
            gj += 1
        s0, s1 = int(gslot[gi]), int(gslot[gj])
        nslots = s1 - s0
        calls[p].append((bb, s0, nslots, idxcol))
        idxcol += nslots * 8  # nslots*128/16 columns
        gi = gj
    assert idxcol == E_struct // 16

    st = Structure(
        TC=TC, E_struct=E_struct, slot_w=slot_w,
        phase_slot0=phase_slot0, phase_slot1=phase_slot1,
        phase_w0=[int(x) for x in w0_of_p], phase_nw=phases,
        calls=calls,
    )

    # per-core staged data
    lidx_pad = np.zeros((C, E_struct), np.int16)
    dwin_pad = np.full((C, E_struct), SENTINEL, np.float32)
    lidx_pad[core, pos] = lidx
    dwin_pad[core, pos] = dwin

    cores = []
    for k in range(C):
        blocks = []
        for p in range(NP):
            for (bb, s0, nslots, _c0) in calls[p]:
                seg = lidx_pad[k, s0 * 128:(s0 + nslots) * 128]
                blocks.append(seg.reshape(-1, 16).T)  # [16, nslots*8]
        blob16 = np.concatenate(blocks, axis=1)
        idx_blob = np.tile(blob16, (8, 1))  # [128, E_struct//16]

        import ml_dtypes
        dstloc = dwin_pad[k].reshape(TC, 128).T.astype(ml_dtypes.bfloat16)

        v = np.ones(NW * WIN, np.float32)
        v[:NPC] = invdeg[k * NPC:(k + 1) * NPC]
        invdegb = np.tile(v, (D, 1))  # [D, NW*WIN]

        ft = np.zeros((D + 1, NW * WIN), np.float32)
        ft[:D, :NPC] = feat[k * NPC:(k + 1) * NPC].T
        ft[D, :] = 1.0
        cores.append(CoreData(idx_blob=idx_blob, dstloc=dstloc,
                              invdegb=invdegb, featT_aug=ft))

    import ml_dtypes
    featb = np.zeros((N, 128), dtype=ml_dtypes.bfloat16)
    featb[:, :D] = feat.astype(ml_dtypes.bfloat16)
    consts = {
        "WselfT_aug": np.concatenate([W_self.T, b[None, :]], 0).astype(np.float32),
        "WneighT": np.ascontiguousarray(W_neigh.T, dtype=np.float32),
        "iota": np.tile(np.arange(128, dtype=ml_dtypes.bfloat16), (128, 1)),
        "featb": featb,
    }
    return st, cores, consts


def build_program(st: Structure, cfg: Cfg = CFG):
    import concourse.bacc as bacc
    import concourse.mybir as mybir
    import concourse.tile as tile

    D, WIN, NW = cfg.D, cfg.WIN, cfg.NW
    NP = len(cfg.PHASE_WINS)
    f32 = mybir.dt.float32
    AL = mybir.AluOpType

    nc = bacc.Bacc("TRN2", target_bir_lowering=False, debug=False,
                   num_swdge_queues=4)

    bf16 = mybir.dt.bfloat16
    feat_d = nc.dram_tensor("featb", [cfg.N, 128], bf16, kind="ExternalInput")
    featT_d = nc.dram_tensor("featT", [D + 1, NW * WIN], f32, kind="ExternalInput")
    idx_d = nc.dram_tensor("idx", [128, st.E_struct // 16], mybir.dt.int16,
                           kind="ExternalInput")
    dstloc_d = nc.dram_tensor("dstloc", [128, st.TC], bf16, kind="ExternalInput")
    invdegb_d = nc.dram_tensor("invdegb", [D, NW * WIN], f32, kind="ExternalInput")
    wself_d = nc.dram_tensor("WselfT_aug", [D + 1, D], f32, kind="ExternalInput")
    wneigh_d = nc.dram_tensor("WneighT", [D, D], f32, kind="ExternalInput")
    iota_d = nc.dram_tensor("iota", [128, 128], bf16, kind="ExternalInput")
    out_d = nc.dram_tensor("out", [NW * WIN, D], f32, kind="ExternalOutput")

    with tile.TileContext(nc) as tc:
        with (
            tc.tile_pool(name="const", bufs=1) as cpool,
            tc.tile_pool(name="arena", bufs=2) as apool,
            tc.tile_pool(name="mask", bufs=2) as mpool,
            tc.tile_pool(name="featT", bufs=2) as fpool,
            tc.tile_pool(name="aggT", bufs=2) as gpool,
            tc.tile_pool(name="invd", bufs=2) as ipool,
            tc.tile_pool(name="outb", bufs=2) as opool,
            tc.tile_pool(name="psA", bufs=2, space="PSUM") as psA,
            tc.tile_pool(name="psO", bufs=2, space="PSUM") as psO,
        ):
            idx_sb = cpool.tile([128, st.E_struct // 16], mybir.dt.int16, tag="idx")
            nc.sync.dma_start(idx_sb[:], idx_d[:, :])
            dstloc_sb = cpool.tile([128, st.TC], bf16, tag="dstloc")
            nc.sync.dma_start(dstloc_sb[:], dstloc_d[:, :])
            iota_sb = cpool.tile([128, 128], bf16, tag="iota")
            nc.sync.dma_start(iota_sb[:], iota_d[:, :])
            wself_sb = cpool.tile([D + 1, D], f32, tag="wself")
            nc.sync.dma_start(wself_sb[:], wself_d[:, :])
            wneigh_sb = cpool.tile([D, D], f32, tag="wneigh")
            nc.sync.dma_start(wneigh_sb[:], wneigh_d[:, :])

            qcounter = [0]
            for p in range(NP):
                ps0, ps1 = st.phase_slot0[p], st.phase_slot1[p]
                tcp = ps1 - ps0          # chunks this phase
                w0, nwv = st.phase_w0[p], st.phase_nw[p]
                n0 = w0 * WIN

                arena = apool.tile([128, tcp, 128], bf16, tag="arena")
                for (bb, s0, nslots, c0) in st.calls[p]:
                    nc.gpsimd.dma_gather(
                        arena[:, s0 - ps0:s0 - ps0 + nslots, :],
                        feat_d[bb * cfg.BROWS:(bb + 1) * cfg.BROWS, :],
                        idx_sb[:, c0:c0 + nslots * 8],
                        nslots * 128,
                        nslots * 128,
                        128,
                        single_packet=False,
                        queue_num=qcounter[0] % 4,
                    )
                    qcounter[0] += 1

                featT_t = fpool.tile([D + 1, nwv * WIN], f32, tag="featT")
                nc.sync.dma_start(featT_t[:], featT_d[:, n0:n0 + nwv * WIN])
                invd_t = ipool.tile([D, nwv * WIN], f32, tag="invd")
                nc.sync.dma_start(invd_t[:], invdegb_d[:, n0:n0 + nwv * WIN])

                # one-hot masks for the whole phase in one DVE op
                mask = mpool.tile([128, tcp, 128], bf16, tag="mask")
                nc.vector.tensor_tensor(
                    mask[:],
                    iota_sb[:].unsqueeze(1).to_broadcast([128, tcp, 128]),
                    dstloc_sb[:, ps0:ps1].unsqueeze(2)
                    .to_broadcast([128, tcp, 128]),
                    AL.is_equal,
                )

                # window-major matmul issue; each window owns one 2KB psum
                # zero region (stride PSUM_WSTRIDE)
                psum_agg = psA.tile([D, nwv * PSUM_WSTRIDE], f32, tag="psA")
                for wl in range(nwv):
                    w = w0 + wl
                    slots = [s for s in range(ps0, ps1) if st.slot_w[s] == w]
                    for i, s in enumerate(slots):
                        nc.tensor.matmul(
                            psum_agg[:, wl * PSUM_WSTRIDE:
                                     wl * PSUM_WSTRIDE + WIN],
                            lhsT=arena[:, s - ps0, 0:D],
                            rhs=mask[:, s - ps0, :],
                            start=(i == 0),
                            stop=(i == len(slots) - 1),
                        )

                # psum -> sbuf with mean normalization: aggT = agg * invdeg
                aggT = gpool.tile([D, nwv * WIN], f32, tag="aggT")
                nc.vector.scalar_tensor_tensor(
                    aggT[:].rearrange("f (w d) -> f w d", d=WIN),
                    psum_agg[:].rearrange("f (w d) -> f w d", d=PSUM_WSTRIDE)
                    [:, :, :WIN],
                    1.0,
                    invd_t[:].rearrange("f (w d) -> f w d", d=WIN),
                    AL.mult,
                    AL.mult,
                )

                outb = opool.tile([128, nwv, D], f32, tag="outb")
                for wl in range(nwv):
                    psum_o = psO.tile([128, D], f32, tag="psO")
                    nc.tensor.matmul(
                        psum_o[:],
                        lhsT=featT_t[:, wl * WIN:(wl + 1) * WIN],
                        rhs=wself_sb[:],
                        start=True, stop=False,
                    )
                    nc.tensor.matmul(
                        psum_o[:],
                        lhsT=aggT[:, wl * WIN:(wl + 1) * WIN],
                        rhs=wneigh_sb[:],
                        start=False, stop=True,
                    )
                    nc.scalar.copy(outb[:, wl, :], psum_o[:])
                nc.sync.dma_start(
                    out_d[n0:n0 + nwv * WIN, :]
                    .rearrange("(w p) d -> p w d", p=128),
                    outb[:],
                )

    nc.compile()
    return nc


def run(feat, src, dst, W_self, W_neigh, b, cfg: Cfg = CFG, trace=False):
    from concourse.bass_utils import run_bass_kernel_spmd

    st, cores, consts = preprocess(feat, src, dst, W_self, W_neigh, b, cfg)
    nc = build_program(st, cfg)

    in_maps = []
    for k in range(cfg.C):
        cd = cores[k]
        in_maps.append({
            "featb": consts["featb"],
            "featT": cd.featT_aug,
            "idx": cd.idx_blob,
            "dstloc": cd.dstloc,
            "invdegb": cd.invdegb,
            "WselfT_aug": consts["WselfT_aug"],
            "WneighT": consts["WneighT"],
            "iota": consts["iota"],
        })
    res = run_bass_kernel_spmd(nc, in_maps, core_ids=list(range(cfg.C)),
                               trace=trace)
    out = np.empty((cfg.N, cfg.D), np.float32)
    for k in range(cfg.C):
        out[k * cfg.NPC:(k + 1) * cfg.NPC] = res.results[k]["out"][:cfg.NPC]
    return out, res


def kernel(feat, src, dst, W_self, W_neigh, b):
    out, _ = run(feat, src, dst, W_self, W_neigh, b)
    return out


# revision 7
# speedup vs baseline: 3.8495x; 1.5263x over previous
"""DistSageConv Trainium2 kernel: SAGE mean-aggregation GNN over 8 NeuronCores.

Strategy: nodes (and their in-edges) are sharded by dst across 8 cores; feat is
replicated in each core's HBM. Each core gathers feat[src] for its edges with
dma_gather (int16 indices -> 4 source banks of 25000 rows), segment-sums them
into its 12500 output nodes via one-hot matmuls on the PE (edges pre-bucketed
by 128-dst windows on host), and applies the dense projections:

    out = feat @ W_self.T + (agg/deg) @ W_neigh.T + b

The per-(window,bank) edge-group sizes are padded to the max across the 8
cores so a single compiled program (SPMD) serves every core.
"""

import numpy as np
from dataclasses import dataclass


@dataclass(frozen=True)
class Cfg:
    N: int = 100000          # nodes
    D: int = 64              # feature dim (in == out)
    C: int = 8               # cores
    WIN: int = 128           # dst window (one-hot width)
    NBANK: int = 4           # src banks (int16 gather index limit)
    PHASE_WINS: tuple = (3,) * 32 + (2,)   # windows per phase (sum == NW)

    @property
    def NPC(self):  # nodes per core
        return self.N // self.C

    @property
    def NW(self):   # windows per core
        return -(-self.NPC // self.WIN)

    @property
    def BROWS(self):  # rows per src bank
        return self.N // self.NBANK


CFG = Cfg()
SENTINEL = 1000.0  # dst-in-window value for padding edges (never matches iota)
PSUM_WSTRIDE = 512  # f32 elems between windows in psum (= one 2KB zero region)


@dataclass
class Structure:
    """Shared (core-independent) program structure."""
    TC: int                      # total chunk slots
    E_struct: int                # padded edge count (TC*128)
    slot_w: np.ndarray           # [TC] global window of each slot
    phase_slot0: list            # per phase: first slot
    phase_slot1: list            # per phase: end slot
    phase_w0: list               # per phase: first window
    phase_nw: list               # per phase: #windows
    calls: list                  # per phase: list of (bank, slot0, nslots, idxcol0)


@dataclass
class CoreData:
    idx_blob: np.ndarray     # [128, E_struct//16] int16
    dstloc: np.ndarray       # [128, TC] f32
    invdegb: np.ndarray      # [D, NW*WIN] f32 (row-replicated 1/deg)
    featT_aug: np.ndarray    # [D+1, NW*WIN] f32  (row D = ones)


def preprocess(feat, src, dst, W_self, W_neigh, b, cfg: Cfg = CFG):
    N, D, C = cfg.N, cfg.D, cfg.C
    NPC, WIN, NW = cfg.NPC, cfg.WIN, cfg.NW
    NBANK, BROWS = cfg.NBANK, cfg.BROWS
    phases = list(cfg.PHASE_WINS)
    NP = len(phases)
    assert sum(phases) == NW

    feat = np.asarray(feat, dtype=np.float32)
    src32 = np.asarray(src).astype(np.int64)
    dst32 = np.asarray(dst).astype(np.int64)
    E = src32.shape[0]
    W_self = np.asarray(W_self, dtype=np.float32)
    W_neigh = np.asarray(W_neigh, dtype=np.float32)
    b = np.asarray(b, dtype=np.float32)

    deg = np.bincount(dst32, minlength=N).astype(np.float32)
    invdeg = (1.0 / np.maximum(deg, 1.0)).astype(np.float32)

    core = dst32 // NPC
    dloc = dst32 - core * NPC
    wl = dloc // WIN                      # window within core
    dwin = (dloc - wl * WIN).astype(np.float32)
    bank = src32 // BROWS
    lidx = (src32 - bank * BROWS).astype(np.int16)

    counts = np.bincount(
        (core * NW + wl) * NBANK + bank, minlength=C * NW * NBANK
    ).reshape(C, NW, NBANK)
    chunks_wb = -(-counts.max(axis=0) // 128)  # [NW, NBANK] ceil

    w0_of_p = np.concatenate([[0], np.cumsum(phases)])[:-1]

    # groups ordered (phase, bank, window); each (phase,bank) run is one
    # dma_gather call writing contiguous arena slots.
    group_list = []
    for p in range(NP):
        for bb in range(NBANK):
            for w in range(w0_of_p[p], w0_of_p[p] + phases[p]):
                if chunks_wb[w, bb] > 0:
                    group_list.append((p, bb, w))
    G = len(group_list)
    gidx = np.full((NW, NBANK), -1, dtype=np.int64)
    for gi, (p, bb, w) in enumerate(group_list):
        gidx[w, bb] = gi
    gchunks = np.array([chunks_wb[w, bb] for (p, bb, w) in group_list])
    gcap = gchunks * 128
    gbase = np.concatenate([[0], np.cumsum(gcap)])[:-1]
    E_struct = int(gcap.sum())
    TC = E_struct // 128

    # padded position of each edge within its core's layout
    gid_e = gidx[wl, bank]
    assert (gid_e >= 0).all()
    keyv = core * G + gid_e
    order = np.argsort(keyv, kind="stable")
    ks = keyv[order]
    starts = np.concatenate([[0], np.flatnonzero(ks[1:] != ks[:-1]) + 1])
    runlen = np.diff(np.concatenate([starts, [E]]))
    rank = np.empty(E, np.int64)
    rank[order] = np.arange(E) - np.repeat(starts, runlen)
    assert (rank < gcap[gid_e]).all()
    pos = gbase[gid_e] + rank

    # slot metadata
    slot_w = np.repeat(np.array([w for (_, _, w) in group_list]), gchunks)
    slot_p = np.repeat(np.array([p for (p, _, _) in group_list]), gchunks)
    phase_slot0 = [int(np.searchsorted(slot_p, p, "left")) for p in range(NP)]
    phase_slot1 = [int(np.searchsorted(slot_p, p, "right")) for p in range(NP)]

    # gather calls: per (phase, bank) contiguous group runs
    calls = [[] for _ in range(NP)]
    idxcol = 0
    gslot = np.concatenate([[0], np.cumsum(gchunks)])  # slot base per group
    gi = 0
    while gi < G:
        p, bb, _ = group_list[gi]
        gj = gi
        while gj < G and group_list[gj][0] == p and group_list[gj][1] == bb:
            gj += 1
        s0, s1 = int(gslot[gi]), int(gslot[gj])
        nslots = s1 - s0
        calls[p].append((bb, s0, nslots, idxcol))
        idxcol += nslots * 8  # nslots*128/16 columns
        gi = gj
    assert idxcol == E_struct // 16

    st = Structure(
        TC=TC, E_struct=E_struct, slot_w=slot_w,
        phase_slot0=phase_slot0, phase_slot1=phase_slot1,
        phase_w0=[int(x) for x in w0_of_p], phase_nw=phases,
        calls=calls,
    )

    # per-core staged data
    lidx_pad = np.zeros((C, E_struct), np.int16)
    dwin_pad = np.full((C, E_struct), SENTINEL, np.float32)
    lidx_pad[core, pos] = lidx
    dwin_pad[core, pos] = dwin

    cores = []
    for k in range(C):
        blocks = []
        for p in range(NP):
            for (bb, s0, nslots, _c0) in calls[p]:
                seg = lidx_pad[k, s0 * 128:(s0 + nslots) * 128]
                blocks.append(seg.reshape(-1, 16).T)  # [16, nslots*8]
        blob16 = np.concatenate(blocks, axis=1)
        idx_blob = np.tile(blob16, (8, 1))  # [128, E_struct//16]

        import ml_dtypes
        dstloc = dwin_pad[k].reshape(TC, 128).T.astype(ml_dtypes.bfloat16)

        v = np.ones(NW * WIN, np.float32)
        v[:NPC] = invdeg[k * NPC:(k + 1) * NPC]
        invdegb = np.tile(v, (D, 1))  # [D, NW*WIN]

        ft = np.zeros((D + 1, NW * WIN), np.float32)
        ft[:D, :NPC] = feat[k * NPC:(k + 1) * NPC].T
        ft[D, :] = 1.0
        cores.append(CoreData(idx_blob=idx_blob, dstloc=dstloc,
                              invdegb=invdegb, featT_aug=ft))

    import ml_dtypes
    featb = np.zeros((N, 128), dtype=ml_dtypes.bfloat16)
    featb[:, :D] = feat.astype(ml_dtypes.bfloat16)
    consts = {
        "WselfT_aug": np.concatenate([W_self.T, b[None, :]], 0).astype(np.float32),
        "WneighT": np.ascontiguousarray(W_neigh.T, dtype=np.float32),
        "iota": np.tile(np.arange(128, dtype=ml_dtypes.bfloat16), (128, 1)),
        "featb": featb,
    }
    return st, cores, consts


def build_program(st: Structure, cfg: Cfg = CFG):
    import concourse.bacc as bacc
    import concourse.mybir as mybir
    import concourse.tile as tile

    D, WIN, NW = cfg.D, cfg.WIN, cfg.NW
    NP = len(cfg.PHASE_WINS)
    f32 = mybir.dt.float32
    AL = mybir.AluOpType

    nc = bacc.Bacc("TRN2", target_bir_lowering=False, debug=False,
                   num_swdge_queues=4)

    bf16 = mybir.dt.bfloat16
    feat_d = nc.dram_tensor("featb", [cfg.N, 128], bf16, kind="ExternalInput")
    featT_d = nc.dram_tensor("featT", [D + 1, NW * WIN], f32, kind="ExternalInput")
    idx_d = nc.dram_tensor("idx", [128, st.E_struct // 16], mybir.dt.int16,
                           kind="ExternalInput")
    dstloc_d = nc.dram_tensor("dstloc", [128, st.TC], bf16, kind="ExternalInput")
    invdegb_d = nc.dram_tensor("invdegb", [D, NW * WIN], f32, kind="ExternalInput")
    wself_d = nc.dram_tensor("WselfT_aug", [D + 1, D], f32, kind="ExternalInput")
    wneigh_d = nc.dram_tensor("WneighT", [D, D], f32, kind="ExternalInput")
    iota_d = nc.dram_tensor("iota", [128, 128], bf16, kind="ExternalInput")
    out_d = nc.dram_tensor("out", [NW * WIN, D], f32, kind="ExternalOutput")

    with tile.TileContext(nc) as tc:
        with (
            tc.tile_pool(name="const", bufs=1) as cpool,
            tc.tile_pool(name="arena", bufs=3) as apool,
            tc.tile_pool(name="mask", bufs=2) as mpool,
            tc.tile_pool(name="featT", bufs=2) as fpool,
            tc.tile_pool(name="aggT", bufs=2) as gpool,
            tc.tile_pool(name="invd", bufs=2) as ipool,
            tc.tile_pool(name="outb", bufs=2) as opool,
            tc.tile_pool(name="psA", bufs=2, space="PSUM") as psA,
            tc.tile_pool(name="psO", bufs=2, space="PSUM") as psO,
        ):
            idx_sb = cpool.tile([128, st.E_struct // 16], mybir.dt.int16, tag="idx")
            nc.sync.dma_start(idx_sb[:], idx_d[:, :])
            dstloc_sb = cpool.tile([128, st.TC], bf16, tag="dstloc")
            nc.sync.dma_start(dstloc_sb[:], dstloc_d[:, :])
            iota_sb = cpool.tile([128, 128], bf16, tag="iota")
            nc.sync.dma_start(iota_sb[:], iota_d[:, :])
            wself_sb = cpool.tile([D + 1, D], f32, tag="wself")
            nc.sync.dma_start(wself_sb[:], wself_d[:, :])
            wneigh_sb = cpool.tile([D, D], f32, tag="wneigh")
            nc.sync.dma_start(wneigh_sb[:], wneigh_d[:, :])

            qcounter = [0]
            for p in range(NP):
                ps0, ps1 = st.phase_slot0[p], st.phase_slot1[p]
                tcp = ps1 - ps0          # chunks this phase
                w0, nwv = st.phase_w0[p], st.phase_nw[p]
                n0 = w0 * WIN

                arena = apool.tile([128, tcp, 128], bf16, tag="arena")
                for (bb, s0, nslots, c0) in st.calls[p]:
                    nc.gpsimd.dma_gather(
                        arena[:, s0 - ps0:s0 - ps0 + nslots, :],
                        feat_d[bb * cfg.BROWS:(bb + 1) * cfg.BROWS, :],
                        idx_sb[:, c0:c0 + nslots * 8],
                        nslots * 128,
                        nslots * 128,
                        128,
                        single_packet=False,
                        queue_num=(qcounter[0] + qcounter[0] // 4) % 4,
                    )
                    qcounter[0] += 1

                featT_t = fpool.tile([D + 1, nwv * WIN], f32, tag="featT")
                nc.sync.dma_start(featT_t[:], featT_d[:, n0:n0 + nwv * WIN])
                invd_t = ipool.tile([D, nwv * WIN], f32, tag="invd")
                nc.sync.dma_start(invd_t[:], invdegb_d[:, n0:n0 + nwv * WIN])

                # one-hot masks for the whole phase in one DVE op
                mask = mpool.tile([128, tcp, 128], bf16, tag="mask")
                nc.vector.tensor_tensor(
                    mask[:],
                    iota_sb[:].unsqueeze(1).to_broadcast([128, tcp, 128]),
                    dstloc_sb[:, ps0:ps1].unsqueeze(2)
                    .to_broadcast([128, tcp, 128]),
                    AL.is_equal,
                )

                # window-major matmul issue; each window owns one 2KB psum
                # zero region (stride PSUM_WSTRIDE)
                psum_agg = psA.tile([D, nwv * PSUM_WSTRIDE], f32, tag="psA")
                for wl in range(nwv):
                    w = w0 + wl
                    slots = [s for s in range(ps0, ps1) if st.slot_w[s] == w]
                    for i, s in enumerate(slots):
                        nc.tensor.matmul(
                            psum_agg[:, wl * PSUM_WSTRIDE:
                                     wl * PSUM_WSTRIDE + WIN],
                            lhsT=arena[:, s - ps0, 0:D],
                            rhs=mask[:, s - ps0, :],
                            start=(i == 0),
                            stop=(i == len(slots) - 1),
                        )

                # psum -> sbuf with mean normalization: aggT = agg * invdeg
                aggT = gpool.tile([D, nwv * WIN], f32, tag="aggT")
                nc.vector.scalar_tensor_tensor(
                    aggT[:].rearrange("f (w d) -> f w d", d=WIN),
                    psum_agg[:].rearrange("f (w d) -> f w d", d=PSUM_WSTRIDE)
                    [:, :, :WIN],
                    1.0,
                    invd_t[:].rearrange("f (w d) -> f w d", d=WIN),
                    AL.mult,
                    AL.mult,
                )

                outb = opool.tile([128, nwv, D], f32, tag="outb")
                for wl in range(nwv):
                    psum_o = psO.tile([128, D], f32, tag="psO")
                    nc.tensor.matmul(
                        psum_o[:],
                        lhsT=featT_t[:, wl * WIN:(wl + 1) * WIN],
                        rhs=wself_sb[:],
                        start=True, stop=False,
                    )
                    nc.tensor.matmul(
                        psum_o[:],
                        lhsT=aggT[:, wl * WIN:(wl + 1) * WIN],
                        rhs=wneigh_sb[:],
                        start=False, stop=True,
                    )
                    nc.scalar.copy(outb[:, wl, :], psum_o[:])
                nc.sync.dma_start(
                    out_d[n0:n0 + nwv * WIN, :]
                    .rearrange("(w p) d -> p w d", p=128),
                    outb[:],
                )

    nc.compile()
    return nc


def run(feat, src, dst, W_self, W_neigh, b, cfg: Cfg = CFG, trace=False):
    from concourse.bass_utils import run_bass_kernel_spmd

    st, cores, consts = preprocess(feat, src, dst, W_self, W_neigh, b, cfg)
    nc = build_program(st, cfg)

    in_maps = []
    for k in range(cfg.C):
        cd = cores[k]
        in_maps.append({
            "featb": consts["featb"],
            "featT": cd.featT_aug,
            "idx": cd.idx_blob,
            "dstloc": cd.dstloc,
            "invdegb": cd.invdegb,
            "WselfT_aug": consts["WselfT_aug"],
            "WneighT": consts["WneighT"],
            "iota": consts["iota"],
        })
    res = run_bass_kernel_spmd(nc, in_maps, core_ids=list(range(cfg.C)),
                               trace=trace)
    out = np.empty((cfg.N, cfg.D), np.float32)
    for k in range(cfg.C):
        out[k * cfg.NPC:(k + 1) * cfg.NPC] = res.results[k]["out"][:cfg.NPC]
    return out, res


def kernel(feat, src, dst, W_self, W_neigh, b):
    out, _ = run(feat, src, dst, W_self, W_neigh, b)
    return out
